# revision 11
# baseline (speedup 1.0000x reference)
"""DKTForgetTotal Trainium2 kernel.

Math reduction (per (b,l,m) with concept s=c[b,l,m] >= 0):
  _forget output = b_pre + sum_m fv1_m @ H[s_m]
  where H_s = W_cemb @ diag(E_skill[s]) @ Wp1_s + Wp2_s  in R^{300x128}
  and fv1_m is 12-hot. Folding the 4 identical rgap one-hots gives a 9-hot
  over a 225-row table H'_s = [sum_f H_s[f*25+v]; H_s[100:200]; H_s[200:300]].

Sharding: concept-space s is split 13/13/13/13/12/12/12/12 across 8 cores.
Each core builds its H' shard on-device from a 2.5MB W_pre slice and
computes partial forget outputs for ALL (b,l) rows via one-hot matmuls
accumulated in PSUM (one PSUM column per (b,l), exploiting that each row's
4 concepts are consecutive so per-core instances form short slot
intervals -> interval coloring with <=128 colors per 4-slot block).
Partials are scatter-added (dup-free) into a DRAM buffer, ReduceScattered,
then everything else (attention, the two LSTMs which scan over B with L as
batch, heads) runs l-sharded (25 positions x 8 batch rows per core).
"""
import sys
sys.path.insert(0, '/opt/trn_rl_repo')

import numpy as np

import concourse.bass as bass
import concourse.mybir as mybir
import concourse.tile as tile
from concourse import bacc
from concourse.bass_utils import run_bass_kernel_spmd
from concourse.masks import make_identity

F32 = mybir.dt.float32
I16 = mybir.dt.int16
I32 = mybir.dt.int32
AF = mybir.ActivationFunctionType
ALU = mybir.AluOpType

N_CORES = 8
B, L, M, S, D, Q = 8, 200, 4, 100, 128, 10000
NRG = NPC = NAC = 20
F = 4
NTOT = 300
LC = L // N_CORES              # 25 l-positions per core
R = B * LC                     # 200 rows per core in the l-sharded stages
NS = 13                        # slot capacity per core (cores 4-7 use 12)
OFFS = [0, 13, 26, 39, 52, 64, 76, 88, 100]
BLK = 4                        # slots per PSUM tile
NTILE = 4                      # ceil(13/4)
NCOL = 128                     # columns per PSUM tile
NPAIR = 20                     # (g, a) matmul pairs per path
H_ROWS = 225                   # 25 rgsum + 100 pc + 100 ac
KC0, KC1 = 125, 100            # K-chunks (seg-aligned: 5*25, 4*25)
WSLOT = 384                    # wpre_sh rows per slot: 128 Wp1 + 225 Wp2' + 31 pad
PT_ROWS = N_CORES * 2 * R      # 3200
PT_FULL = PT_ROWS + 128        # + dustbin rows
NSCAT = NTILE * NCOL           # 512 staged rows per path
ISQD = float(1.0 / np.sqrt(D))

# pair table: (g, a). a=0: home tile g//4; a=1: home tile g//4-1.
PAIRS = [(g, 0) for g in range(NS)] + \
        [(g, 1) for g in range(NS) if g // BLK >= 1 and g % BLK <= 2]
assert len(PAIRS) == NPAIR
PAIR_TILE = [(g // BLK - a) for (g, a) in PAIRS]
# emission order: all pairs of tile 0, then tile 1, ... (PSUM-group consecutive)
PAIR_ORDER = [i for tt in range(NTILE) for i in range(NPAIR) if PAIR_TILE[i] == tt]

_CACHE = {}


def _wrap_idx(idx, n_pad):
    """int16 index list -> [128, n_pad//16] wrapped/replicated layout."""
    assert len(idx) <= n_pad
    a = np.full(n_pad, -1, np.int16)
    a[:len(idx)] = idx
    a = a.reshape(n_pad // 16, 16).T          # [16, n/16]
    return np.tile(a, (8, 1)).astype(np.int16)  # replicate to 128 partitions


def _host_prep(inp):
    """Build the per-core input maps (pure numpy index/layout prep)."""
    E_skill = np.asarray(inp['E_skill'], np.float32)
    W_cemb = np.asarray(inp['W_cemb'], np.float32)
    W_pre = np.asarray(inp['W_pre'], np.float32).reshape(S, D + NTOT, D)
    E_q = np.ascontiguousarray(np.asarray(inp['E_question'], np.float32))
    E_r = np.asarray(inp['E_response'], np.float32)

    # W'_cemb = [rgsum(25); pc rows; ac rows]  [225, 128]
    wc_rg = W_cemb[0:100].reshape(F, 25, D).sum(0)
    wc_p = np.concatenate([wc_rg, W_cemb[100:200], W_cemb[200:300]], 0)
    wcembT = np.zeros((D, 256), np.float32)
    wcembT[:, :H_ROWS] = wc_p.T

    paths = []
    for pi, (cn, rn, pn, an) in enumerate([
            ('c', 'rgaps', 'pcounts', 'acounts'),
            ('shift_c', 'shft_rgaps', 'shft_pcounts', 'shft_acounts')]):
        paths.append((np.asarray(inp[cn], np.int64),
                      np.asarray(inp[rn], np.int64),
                      np.asarray(inp[pn], np.int64),
                      np.asarray(inp[an], np.int64)))

    in_maps = []
    for k in range(N_CORES):
        s_lo, s_hi = OFFS[k], OFFS[k + 1]
        nsk = s_hi - s_lo
        im = {}
        # --- wpre shard [13, 384, 128] ---
        wsh = np.zeros((NS, WSLOT, D), np.float32)
        for i in range(nsk):
            s = s_lo + i
            blk = W_pre[s]                      # [428, 128]
            wsh[i, 0:128] = blk[0:128]
            wp2 = blk[128:428]                  # [300, 128]
            wsh[i, 128:153] = wp2[0:100].reshape(F, 25, D).sum(0)
            wsh[i, 153:253] = wp2[100:200]
            wsh[i, 253:353] = wp2[200:300]
        im['wpre_sh'] = wsh
        im['wcembT'] = wcembT
        esh = np.zeros((D, NS), np.float32)
        esh[:, :nsk] = E_skill[s_lo:s_hi].T
        im['eskT_sh'] = esh

        # --- forget instance assignment for this core ---
        vals = np.full((9, 2, NPAIR, NCOL), -1.0, np.float32)
        scat = np.full((2, NSCAT), PT_ROWS, np.int64)  # dustbin default
        for pi in range(2):
            c, rg, pc, ac = paths[pi]
            valid = (c >= s_lo) & (c < s_hi)
            bi, li, mi = np.nonzero(valid)
            ss = c[bi, li, mi] - s_lo           # slot in [0, nsk)
            # interval per (b,l)
            key = bi * L + li
            order = np.argsort(key, kind='stable')
            # home tile / coloring
            from collections import defaultdict
            inst_by_bl = defaultdict(list)
            for j in order:
                inst_by_bl[key[j]].append(j)
            # one PSUM column per (b,l) row, allocated per home tile for the
            # whole path (the accumulator is only flushed once per path, so
            # columns are never reused)
            nextcol = [0] * NTILE
            bls = sorted(inst_by_bl.keys(),
                         key=lambda kk: min(ss[j] for j in inst_by_bl[kk]))
            color = {}
            home = {}
            for kk in bls:
                js = inst_by_bl[kk]
                lo = min(ss[j] for j in js)
                hi = max(ss[j] for j in js)
                t = lo // BLK
                assert hi - lo <= 3
                col = nextcol[t]
                nextcol[t] += 1
                assert col < NCOL, f"core {k} path {pi} tile {t}: out of columns"
                color[kk] = col
                home[kk] = t
                # scatter target for this column
                b = kk // L
                l = kk % L
                row = (l // LC) * (2 * R) + pi * R + b * LC + (l % LC)
                scat[pi, t * NCOL + col] = row
            # fill vals per instance
            pair_idx = {p: i for i, p in enumerate(PAIRS)}
            for kk in bls:
                for j in inst_by_bl[kk]:
                    g = int(ss[j])
                    a = 0 if g // BLK == home[kk] else 1
                    pidx = pair_idx[(g, a)]
                    col = color[kk]
                    assert vals[0, pi, pidx, col] < 0, "column collision"
                    vals[0, pi, pidx, col] = rg[bi[j], li[j], mi[j]]
                    vals[1:5, pi, pidx, col] = pc[bi[j], li[j], mi[j]]
                    vals[5:9, pi, pidx, col] = ac[bi[j], li[j], mi[j]]
        im['valsT'] = vals.reshape(9, 2 * NPAIR * NCOL)
        im['scat0'] = _wrap_idx(scat[0].astype(np.int16), NSCAT)
        im['scat1'] = _wrap_idx(scat[1].astype(np.int16), NSCAT)

        # --- l-sharded inputs ---
        lsl = slice(k * LC, (k + 1) * LC)
        q = np.asarray(inp['q'], np.int64)[:, lsl].reshape(-1)        # [200] b-major
        sq = np.asarray(inp['shift_q'], np.int64)[:, lsl].reshape(-1)
        qi = np.concatenate([q, sq]).astype(np.int16)
        im['qidx'] = _wrap_idx(qi, 512)
        r = np.asarray(inp['r'], np.int64)[:, lsl].reshape(-1)
        im['r2'] = np.stack([r.astype(np.float32),
                             np.ones(R, np.float32)])                 # [2, 200]
        cc = np.full((256, 8), -5.0, np.float32)
        for pi in range(2):
            c = paths[pi][0][:, lsl]                                  # [B, LC, M]
            cc[:R, pi * 4:(pi + 1) * 4] = c.reshape(R, M).astype(np.float32)
        im['ccolsT'] = cc

        # patt columns
        p0 = np.zeros((KC0, 1), np.float32)
        p0[:, 0] = np.arange(KC0) % 25
        p1 = np.zeros((KC1, 1), np.float32)
        p1[:, 0] = np.arange(KC1) % 25
        im['patt0'] = p0
        im['patt1'] = p1

        # --- replicated params ---
        im['eq'] = E_q
        im['eresp2'] = np.stack([E_r[1] - E_r[0], E_r[0]])            # [2, 128]
        im['eskT'] = np.ascontiguousarray(E_skill.T)                  # [128, 100]
        im['esk'] = E_skill                                           # [100, 128]
        im['wl2'] = np.asarray(inp['W_l2'], np.float32)               # [384,128]
        im['wl3'] = np.asarray(inp['W_l3'], np.float32)
        im['wfc4'] = np.asarray(inp['W_fc4'], np.float32)
        im['wfc5'] = np.asarray(inp['W_fc5'], np.float32)
        im['wfc3'] = np.asarray(inp['W_fc3'], np.float32)             # [256,1]
        for nm, wn in [('bl2c', 'b_l2'), ('bl3c', 'b_l3'),
                       ('bfc4c', 'b_fc4'), ('bfc5c', 'b_fc5'),
                       ('bprec', 'b_pre')]:
            im[nm] = np.asarray(inp[wn], np.float32).reshape(D, 1)
        im['bfc3'] = np.asarray(inp['b_fc3'], np.float32).reshape(1, 1)
        for nm, wn in [('wihT_in', 'wih_in'), ('whhT_in', 'whh_in'),
                       ('wihT_fg', 'wih_fg'), ('whhT_fg', 'whh_fg')]:
            im[nm] = np.ascontiguousarray(np.asarray(inp[wn], np.float32).T)  # [128, 512]
        im['bg_in'] = np.ascontiguousarray(
            (np.asarray(inp['bih_in'], np.float32)
             + np.asarray(inp['bhh_in'], np.float32)).reshape(4, D).T)  # [128, 4]
        im['bg_fg'] = np.ascontiguousarray(
            (np.asarray(inp['bih_fg'], np.float32)
             + np.asarray(inp['bhh_fg'], np.float32)).reshape(4, D).T)
        in_maps.append(im)
    return in_maps


def _declare_inputs(nc):
    t = {}
    def di(name, shape, dtype=F32):
        t[name] = nc.dram_tensor(name, list(shape), dtype, kind="ExternalInput")
    di('wpre_sh', (NS, WSLOT, D)); di('wcembT', (D, 256)); di('eskT_sh', (D, NS))
    di('valsT', (9, 2 * NPAIR * NCOL))
    di('scat0', (128, NSCAT // 16), I16); di('scat1', (128, NSCAT // 16), I16)
    di('qidx', (128, 512 // 16), I16)
    di('r2', (2, R)); di('ccolsT', (256, 8))
    di('patt0', (KC0, 1)); di('patt1', (KC1, 1))
    di('eq', (Q, D)); di('eresp2', (2, D)); di('eskT', (D, S)); di('esk', (S, D))
    di('wl2', (3 * D, D)); di('wl3', (2 * D, D))
    di('wfc4', (2 * D, D)); di('wfc5', (2 * D, D)); di('wfc3', (2 * D, 1))
    for nm in ['bl2c', 'bl3c', 'bfc4c', 'bfc5c', 'bprec']:
        di(nm, (D, 1))
    di('bfc3', (1, 1))
    for nm in ['wihT_in', 'whhT_in', 'wihT_fg', 'whhT_fg']:
        di(nm, (D, 4 * D))
    di('bg_in', (D, 4)); di('bg_fg', (D, 4))
    return t


def _build_module(debug=False):
    nc = bacc.Bacc("TRN2", target_bir_lowering=False, debug=False,
                   num_devices=N_CORES)
    t = _declare_inputs(nc)
    y_out = nc.dram_tensor("y", [1, R], F32, kind="ExternalOutput")
    pt = nc.dram_tensor("pt", [PT_FULL, D], F32)
    ptr = nc.dram_tensor("ptr", [2 * R, D], F32)
    dbg = {}
    if debug:
        for nm, w in [('d_fh', 2 * R), ('d_xemb', R), ('d_att', 2 * R),
                      ('d_ih', R), ('d_fgh', R), ('d_x1', R), ('d_x2', R)]:
            dbg[nm] = nc.dram_tensor(nm, [D, w], F32, kind="ExternalOutput")

    with tile.TileContext(nc) as tc:
        _emit(nc, tc, t, y_out, pt, ptr, dbg)
    nc.compile()
    return nc


def _emit(nc, tc, t, y_out, pt, ptr, dbg=None):
    import contextlib
    ctx = contextlib.ExitStack()
    cp = ctx.enter_context(tc.tile_pool(name="const", bufs=1))
    wp = ctx.enter_context(tc.tile_pool(name="work", bufs=2))
    pp = ctx.enter_context(tc.tile_pool(name="psum", bufs=4, space="PSUM"))
    ppU = ctx.enter_context(tc.tile_pool(name="psumU", bufs=1, space="PSUM"))

    ident = cp.tile([128, 128], F32, tag="ident")
    make_identity(nc, ident[:])

    # ---------------- zero Pt ----------------
    zt = cp.tile([128, D], F32, tag="zt")
    nc.vector.memset(zt[:], 0.0)
    ptv = pt[:].rearrange("(a p) d -> p a d", p=128)          # [128, 26, 128]
    ztb = zt[:].rearrange("p (a d) -> p a d", a=1).to_broadcast([128, PT_FULL // 128, D])
    nc.sync.dma_start(out=ptv, in_=ztb)

    # ---------------- load constants ----------------
    def load(name, shape=None, dtype=F32, tag=None):
        tl = cp.tile(shape or list(t[name].shape), dtype, tag=tag or name)
        nc.sync.dma_start(out=tl[:], in_=t[name][:])
        return tl

    wcembT = load('wcembT')
    eskT_sh = load('eskT_sh')
    eskT = load('eskT')
    esk = load('esk')
    eresp2 = load('eresp2')
    r2 = load('r2')
    ccolsT = cp.tile([128, 2, 8], F32, tag="ccolsT")
    nc.sync.dma_start(out=ccolsT[:],
                      in_=t['ccolsT'][:].rearrange("(a p) c -> p a c", p=128))
    patt0 = load('patt0')
    patt1 = load('patt1')
    def load_kchunks(name, nchunk, ncol=D):
        tl = cp.tile([128, nchunk, ncol], F32, tag=name)
        nc.sync.dma_start(
            out=tl[:], in_=t[name][:].rearrange("(a p) d -> p a d", p=128))
        return tl
    wl2 = load_kchunks('wl2', 3)
    wl3 = load_kchunks('wl3', 2)
    wfc4 = load_kchunks('wfc4', 2)
    wfc5 = load_kchunks('wfc5', 2)
    wfc3 = load_kchunks('wfc3', 2, ncol=1)
    bl2c = load('bl2c'); bl3c = load('bl3c')
    bfc4c = load('bfc4c'); bfc5c = load('bfc5c'); bprec = load('bprec')
    bfc3 = load('bfc3')
    wihT_in = load('wihT_in'); whhT_in = load('whhT_in')
    wihT_fg = load('wihT_fg'); whhT_fg = load('whhT_fg')
    bg_in = load('bg_in'); bg_fg = load('bg_fg')
    scat_idx = [load('scat0', dtype=I16), load('scat1', dtype=I16)]
    qidx = load('qidx', dtype=I16)

    # Wp1 [128, 13, 128] and Wp2' chunks [125|100, 13, 128] from wpre_sh
    wsh = t['wpre_sh']
    wp1 = cp.tile([128, NS, 128], F32, tag="wp1")
    nc.sync.dma_start(
        out=wp1[:],
        in_=wsh[:].rearrange("s p d -> p s d")[0:128])
    wp2a = cp.tile([KC0, NS, 128], F32, tag="wp2a")
    nc.sync.dma_start(
        out=wp2a[:],
        in_=wsh[:].rearrange("s p d -> p s d")[128:128 + KC0])
    wp2b = cp.tile([KC1, NS, 128], F32, tag="wp2b")
    nc.sync.dma_start(
        out=wp2b[:],
        in_=wsh[:].rearrange("s p d -> p s d")[128 + KC0:128 + H_ROWS])

    # ---------------- H' build ----------------
    a_sb = cp.tile([128, NS, 128], F32, tag="a_sb")
    for i in range(NS):
        nc.vector.tensor_scalar(
            out=a_sb[:, i, :], in0=wp1[:, i, :],
            scalar1=eskT_sh[:, i:i + 1], scalar2=None, op0=ALU.mult)
    h0 = cp.tile([KC0, NS, 128], F32, tag="h0")
    h1 = cp.tile([KC1, NS, 128], F32, tag="h1")
    for i in range(NS):
        ps0 = pp.tile([KC0, 128], F32, tag="ps")
        nc.tensor.matmul(out=ps0[:], lhsT=wcembT[:, 0:KC0], rhs=a_sb[:, i, :],
                         start=True, stop=True)
        nc.vector.tensor_tensor(out=h0[:, i, :], in0=ps0[:], in1=wp2a[:, i, :],
                                op=ALU.add)
        ps1 = pp.tile([KC1, 128], F32, tag="ps")
        nc.tensor.matmul(out=ps1[:], lhsT=wcembT[:, KC0:H_ROWS], rhs=a_sb[:, i, :],
                         start=True, stop=True)
        nc.vector.tensor_tensor(out=h1[:, i, :], in0=ps1[:], in1=wp2b[:, i, :],
                                op=ALU.add)

    # ---------------- one-hot (transposed) build ----------------
    # VALT chunk tiles: [125|100, 2*NPAIR*NCOL] via 9 partition-bcast DMAs
    valt0 = cp.tile([KC0, 2 * NPAIR * NCOL], F32, tag="valt0")
    valt1 = cp.tile([KC1, 2 * NPAIR * NCOL], F32, tag="valt1")
    for j in range(5):
        nc.sync.dma_start(out=valt0[25 * j:25 * (j + 1), :],
                          in_=t['valsT'][j:j + 1, :].to_broadcast([25, 2 * NPAIR * NCOL]))
    for j in range(4):
        nc.sync.dma_start(out=valt1[25 * j:25 * (j + 1), :],
                          in_=t['valsT'][5 + j:6 + j, :].to_broadcast([25, 2 * NPAIR * NCOL]))
    oht0 = cp.tile([KC0, 2 * NPAIR * NCOL], F32, tag="oht0")
    oht1 = cp.tile([KC1, 2 * NPAIR * NCOL], F32, tag="oht1")
    nc.vector.tensor_scalar(out=oht0[:], in0=valt0[:], scalar1=patt0[:, 0:1],
                            scalar2=None, op0=ALU.is_equal)
    nc.gpsimd.tensor_scalar(out=oht1[:], in0=valt1[:], scalar1=patt1[:, 0:1],
                            scalar2=None, op0=ALU.is_equal)

    # ---------------- group matmuls into U PSUM tiles ----------------
    u_ps = [ppU.tile([NCOL, NTILE, 128], F32, tag=f"u{pi}", name=f"u{pi}")
            for pi in range(2)]
    stag = [wp.tile([128, NTILE, 128], F32, tag=f"stag{pi}", name=f"stag{pi}")
            for pi in range(2)]
    for pi in range(2):
        for j, i in enumerate(PAIR_ORDER):
            g, a = PAIRS[i]
            tt = PAIR_TILE[i]
            col0 = (pi * NPAIR + i) * NCOL
            first = (j == 0 or PAIR_TILE[PAIR_ORDER[j - 1]] != tt)
            last = (j == NPAIR - 1 or PAIR_TILE[PAIR_ORDER[j + 1]] != tt)
            nc.tensor.matmul(out=u_ps[pi][:, tt, :],
                             lhsT=oht0[:, col0:col0 + NCOL],
                             rhs=h0[:, g, :], start=first, stop=False,
                             skip_group_check=True)
            nc.tensor.matmul(out=u_ps[pi][:, tt, :],
                             lhsT=oht1[:, col0:col0 + NCOL],
                             rhs=h1[:, g, :], start=False, stop=last,
                             skip_group_check=True)
        nc.vector.tensor_copy(out=stag[pi][:], in_=u_ps[pi][:])
        nc.gpsimd.dma_scatter_add(
            out_ap=pt[:], in_ap=stag[pi][:], idxs_ap=scat_idx[pi][:],
            num_idxs=NSCAT, num_idxs_reg=NSCAT, elem_size=D)

    # ---------------- ReduceScatter ----------------
    nc.gpsimd.collective_compute(
        "ReduceScatter", ALU.add,
        replica_groups=[list(range(N_CORES))],
        ins=[pt[0:PT_ROWS, :]], outs=[ptr[:]])

    # fhT [128, 400]: cols 0:200 = forget path, 200:400 = shift path (b-major)
    fhT = cp.tile([128, 2 * R], F32, tag="fhT")
    for jj in range(4):
        pc_t = wp.tile([100, D], F32, tag="ptr_l")
        nc.sync.dma_start(out=pc_t[:], in_=ptr[100 * jj:100 * (jj + 1), :])
        tps = pp.tile([128, 100], F32, tag="ps")
        nc.tensor.transpose(out=tps[:], in_=pc_t[:], identity=ident[:100, :100])
        nc.vector.tensor_scalar(out=fhT[:, 100 * jj:100 * (jj + 1)], in0=tps[:],
                                scalar1=bprec[:, 0:1], scalar2=None, op0=ALU.add)

    if dbg:
        nc.sync.dma_start(out=dbg['d_fh'][:], in_=fhT[:])

    # ---------------- E_question gather + transpose ----------------
    embg = wp.tile([128, 4, D], F32, tag="embg")
    nc.gpsimd.dma_gather(out_ap=embg[:], in_ap=t['eq'][:], idxs_ap=qidx[:],
                         num_idxs=512, num_idxs_reg=2 * R, elem_size=D)
    embT = cp.tile([128, 512], F32, tag="embT")
    for jj in range(4):
        tps = pp.tile([128, 128], F32, tag="ps")
        nc.tensor.transpose(out=tps[:], in_=embg[:, jj, :], identity=ident[:])
        nc.vector.tensor_copy(out=embT[:, 128 * jj:128 * (jj + 1)], in_=tps[:])
    # emb_qT = embT[:, 0:200], shift_qT = embT[:, 200:400]

    # emb_rT [128, 200] = eresp2.T @ r2
    embrT = cp.tile([128, R], F32, tag="embrT")
    psr = pp.tile([128, R], F32, tag="ps")
    nc.tensor.matmul(out=psr[:], lhsT=eresp2[:], rhs=r2[:], start=True, stop=True)
    nc.vector.tensor_copy(out=embrT[:], in_=psr[:])

    # ---------------- attention (both paths) ----------------
    iotai = cp.tile([128, S], I32, tag="iotai")
    nc.gpsimd.iota(iotai[:], pattern=[[1, S]], base=0, channel_multiplier=0)
    iotf = cp.tile([128, S], F32, tag="iotf")
    nc.vector.tensor_copy(out=iotf[:], in_=iotai[:])
    attT = {}
    for pi in range(2):
        qT = embT[:, pi * R:(pi + 1) * R]
        wexpT = wp.tile([S, R], F32, tag=f"wexpT{pi}")
        for ch, (r0, nr) in enumerate([(0, 128), (128, 72)]):
            scps = pp.tile([128, S], F32, tag="ps")
            nc.tensor.matmul(out=scps[:nr], lhsT=qT[:, r0:r0 + nr],
                             rhs=eskT[:], start=True, stop=True)
            cv1 = wp.tile([128, S], F32, tag="cv1")
            for mm in range(M):
                cm = ccolsT[:nr, ch, 4 * pi + mm:4 * pi + mm + 1]
                if mm == 0:
                    nc.vector.tensor_scalar(out=cv1[:nr], in0=iotf[:nr],
                                            scalar1=cm, scalar2=None,
                                            op0=ALU.is_equal)
                else:
                    tmpm = wp.tile([128, S], F32, tag="tmpm")
                    nc.vector.tensor_scalar(out=tmpm[:nr], in0=iotf[:nr],
                                            scalar1=cm, scalar2=None,
                                            op0=ALU.is_equal)
                    nc.vector.tensor_tensor(out=cv1[:nr], in0=cv1[:nr],
                                            in1=tmpm[:nr], op=ALU.add)
            sc = wp.tile([128, S], F32, tag="sc")
            nc.vector.tensor_tensor(out=sc[:nr], in0=scps[:nr], in1=cv1[:nr],
                                    op=ALU.mult)
            ex = wp.tile([128, S], F32, tag="ex")
            nc.scalar.activation(out=ex[:nr], in_=sc[:nr], func=AF.Exp,
                                 scale=ISQD)
            den = wp.tile([128, 1], F32, tag="den")
            nc.vector.tensor_reduce(out=den[:nr], in_=ex[:nr],
                                    axis=mybir.AxisListType.X, op=ALU.add)
            rec = wp.tile([128, 1], F32, tag="rec")
            nc.vector.reciprocal(out=rec[:nr], in_=den[:nr])
            wexp = wp.tile([128, S], F32, tag="wexp")
            nc.vector.tensor_tensor(out=wexp[:nr], in0=ex[:nr], in1=cv1[:nr],
                                    op=ALU.mult)
            nc.vector.tensor_scalar(out=wexp[:nr], in0=wexp[:nr],
                                    scalar1=rec[:nr, 0:1], scalar2=None,
                                    op0=ALU.mult)
            wps = pp.tile([S, 128], F32, tag="ps")
            nc.tensor.transpose(out=wps[:, :nr], in_=wexp[:nr],
                                identity=ident[:nr, :nr])
            nc.vector.tensor_copy(out=wexpT[:, r0:r0 + nr], in_=wps[:, :nr])
        aT = cp.tile([128, R], F32, tag=f"attT{pi}")
        aps = pp.tile([128, R], F32, tag="ps")
        nc.tensor.matmul(out=aps[:], lhsT=esk[:], rhs=wexpT[:], start=True,
                         stop=True)
        nc.vector.tensor_copy(out=aT[:], in_=aps[:])
        attT[pi] = aT

    # ---------------- xembT / shiftT ----------------
    xembT = cp.tile([128, R], F32, tag="xembT")
    xps = pp.tile([128, R], F32, tag="ps")
    nc.tensor.matmul(out=xps[:], lhsT=wl2[:, 0, :], rhs=embT[:, 0:R],
                     start=True, stop=False)
    nc.tensor.matmul(out=xps[:], lhsT=wl2[:, 1, :], rhs=embrT[:],
                     start=False, stop=False)
    nc.tensor.matmul(out=xps[:], lhsT=wl2[:, 2, :], rhs=attT[0][:],
                     start=False, stop=True)
    nc.vector.tensor_scalar(out=xembT[:], in0=xps[:], scalar1=bl2c[:, 0:1],
                            scalar2=None, op0=ALU.add)

    if dbg:
        nc.sync.dma_start(out=dbg['d_xemb'][:], in_=xembT[:])
        nc.sync.dma_start(out=dbg['d_att'][:, 0:R], in_=attT[0][:])
        nc.sync.dma_start(out=dbg['d_att'][:, R:2 * R], in_=attT[1][:])

    shiftT = cp.tile([128, R], F32, tag="shiftT")
    sps = pp.tile([128, R], F32, tag="ps")
    nc.tensor.matmul(out=sps[:], lhsT=wl3[:, 0, :], rhs=embT[:, R:2 * R],
                     start=True, stop=False)
    nc.tensor.matmul(out=sps[:], lhsT=wl3[:, 1, :], rhs=attT[1][:],
                     start=False, stop=True)
    nc.vector.tensor_scalar(out=shiftT[:], in0=sps[:], scalar1=bl3c[:, 0:1],
                            scalar2=None, op0=ALU.add)

    # ---------------- the two LSTMs (scan over B, batch = LC) ----------------
    def lstm(xT, wihT, whhT, bg, name):
        """xT [128, 200] (cols b-major: b*LC+l). Returns hT [128, 200]."""
        gih = cp.tile([128, 4, R], F32, tag=f"gih_{name}")
        for gi in range(4):
            gps = pp.tile([128, R], F32, tag="ps")
            nc.tensor.matmul(out=gps[:], lhsT=wihT[:, gi * 128:(gi + 1) * 128],
                             rhs=xT[:], start=True, stop=True)
            nc.vector.tensor_scalar(out=gih[:, gi, :], in0=gps[:],
                                    scalar1=bg[:, gi:gi + 1], scalar2=None,
                                    op0=ALU.add)
        hT = cp.tile([128, R], F32, tag=f"hT_{name}")
        h = wp.tile([128, LC], F32, tag=f"h_{name}")
        cs = wp.tile([128, LC], F32, tag=f"cs_{name}")
        nc.vector.memset(h[:], 0.0)
        nc.vector.memset(cs[:], 0.0)
        for b in range(B):
            sl = slice(b * LC, (b + 1) * LC)
            gps = pp.tile([128, 4, LC], F32, tag="ps")
            for gi in range(4):
                nc.tensor.matmul(out=gps[:, gi, :],
                                 lhsT=whhT[:, gi * 128:(gi + 1) * 128],
                                 rhs=h[:], start=True, stop=True)
            gt = wp.tile([128, 4, LC], F32, tag=f"gt_{name}")
            nc.vector.tensor_tensor(out=gt[:], in0=gps[:],
                                    in1=gih[:, :, sl], op=ALU.add)
            sig = wp.tile([128, 4, LC], F32, tag=f"sig_{name}")
            nc.scalar.activation(out=sig[:, 0, :], in_=gt[:, 0, :], func=AF.Sigmoid)
            nc.scalar.activation(out=sig[:, 1, :], in_=gt[:, 1, :], func=AF.Sigmoid)
            nc.scalar.activation(out=sig[:, 2, :], in_=gt[:, 2, :], func=AF.Tanh)
            nc.scalar.activation(out=sig[:, 3, :], in_=gt[:, 3, :], func=AF.Sigmoid)
            cs2 = wp.tile([128, LC], F32, tag=f"cs2_{name}")
            nc.vector.tensor_tensor(out=cs2[:], in0=sig[:, 1, :], in1=cs[:],
                                    op=ALU.mult)
            it = wp.tile([128, LC], F32, tag=f"it_{name}")
            nc.vector.tensor_tensor(out=it[:], in0=sig[:, 0, :], in1=sig[:, 2, :],
                                    op=ALU.mult)
            nc.vector.tensor_tensor(out=cs[:], in0=cs2[:], in1=it[:], op=ALU.add)
            tc_t = wp.tile([128, LC], F32, tag=f"tc_{name}")
            nc.scalar.activation(out=tc_t[:], in_=cs[:], func=AF.Tanh)
            nc.vector.tensor_tensor(out=h[:], in0=sig[:, 3, :], in1=tc_t[:],
                                    op=ALU.mult)
            nc.vector.tensor_copy(out=hT[:, sl], in_=h[:])
        return hT

    inputhT = lstm(xembT, wihT_in, whhT_in, bg_in, "in")
    forgethT = lstm(fhT[:, 0:R], wihT_fg, whhT_fg, bg_fg, "fg")

    if dbg:
        nc.sync.dma_start(out=dbg['d_ih'][:], in_=inputhT[:])
        nc.sync.dma_start(out=dbg['d_fgh'][:], in_=forgethT[:])

    # ---------------- heads ----------------
    x1T = cp.tile([128, R], F32, tag="x1T")
    h1ps = pp.tile([128, R], F32, tag="ps")
    nc.tensor.matmul(out=h1ps[:], lhsT=wfc4[:, 0, :], rhs=shiftT[:],
                     start=True, stop=False)
    nc.tensor.matmul(out=h1ps[:], lhsT=wfc4[:, 1, :], rhs=inputhT[:],
                     start=False, stop=True)
    nc.scalar.activation(out=x1T[:], in_=h1ps[:], func=AF.Relu,
                         bias=bfc4c[:, 0:1])
    x2T = cp.tile([128, R], F32, tag="x2T")
    h2ps = pp.tile([128, R], F32, tag="ps")
    nc.tensor.matmul(out=h2ps[:], lhsT=wfc5[:, 0, :], rhs=fhT[:, R:2 * R],
                     start=True, stop=False)
    nc.tensor.matmul(out=h2ps[:], lhsT=wfc5[:, 1, :], rhs=forgethT[:],
                     start=False, stop=True)
    nc.scalar.activation(out=x2T[:], in_=h2ps[:], func=AF.Relu,
                         bias=bfc5c[:, 0:1])

    if dbg:
        nc.sync.dma_start(out=dbg['d_x1'][:], in_=x1T[:])
        nc.sync.dma_start(out=dbg['d_x2'][:], in_=x2T[:])

    yps = pp.tile([1, R], F32, tag="ps")
    nc.tensor.matmul(out=yps[:], lhsT=wfc3[:, 0, :], rhs=x1T[:],
                     start=True, stop=False)
    nc.tensor.matmul(out=yps[:], lhsT=wfc3[:, 1, :], rhs=x2T[:],
                     start=False, stop=True)
    ysb = wp.tile([1, R], F32, tag="ysb")
    nc.scalar.activation(out=ysb[:], in_=yps[:], func=AF.Sigmoid,
                         bias=bfc3[0:1, 0:1])
    nc.sync.dma_start(out=y_out[:], in_=ysb[:])
    ctx.close()


def get_module(debug=False):
    key = ('ncd' if debug else 'nc')
    if key not in _CACHE:
        _CACHE[key] = _build_module(debug)
    return _CACHE[key]


def kernel_debug(**inputs):
    nc = get_module(debug=True)
    in_maps = _host_prep(inputs)
    res = run_bass_kernel_spmd(nc, in_maps, list(range(N_CORES)), trace=False)
    return res.results


def kernel(**inputs):
    nc = get_module()
    in_maps = _host_prep(inputs)
    res = run_bass_kernel_spmd(nc, in_maps, list(range(N_CORES)), trace=False)
    y = np.zeros((B, L), np.float32)
    for k in range(N_CORES):
        y[:, k * LC:(k + 1) * LC] = res.results[k]['y'].reshape(B, LC)
    return y


# revision 18
# speedup vs baseline: 1.4308x; 1.4308x over previous
"""DKTForgetTotal Trainium2 kernel.

Math reduction (per (b,l,m) with concept s=c[b,l,m] >= 0):
  _forget output = b_pre + sum_m fv1_m @ H[s_m]
  where H_s = W_cemb @ diag(E_skill[s]) @ Wp1_s + Wp2_s  in R^{300x128}
  and fv1_m is 12-hot. Folding the 4 identical rgap one-hots gives a 9-hot
  over a 225-row table H'_s = [sum_f H_s[f*25+v]; H_s[100:200]; H_s[200:300]].

Sharding: concept-space s is split 13/13/13/13/12/12/12/12 across 8 cores.
Each core builds its H' shard on-device from a 2.5MB W_pre slice and
computes partial forget outputs for ALL (b,l) rows via one-hot matmuls
accumulated in PSUM (one PSUM column per (b,l), exploiting that each row's
4 concepts are consecutive so per-core instances form short slot
intervals -> interval coloring with <=128 colors per 4-slot block).
Partials are scatter-added (dup-free) into a DRAM buffer, ReduceScattered,
then everything else (attention, the two LSTMs which scan over B with L as
batch, heads) runs l-sharded (25 positions x 8 batch rows per core).
"""
import sys
sys.path.insert(0, '/opt/trn_rl_repo')

import numpy as np

import concourse.bass as bass
import concourse.mybir as mybir
import concourse.tile as tile
from concourse import bacc
from concourse.bass_utils import run_bass_kernel_spmd
from concourse.masks import make_identity

F32 = mybir.dt.float32
BF16 = mybir.dt.bfloat16
I16 = mybir.dt.int16
I32 = mybir.dt.int32
AF = mybir.ActivationFunctionType
ALU = mybir.AluOpType

N_CORES = 8
B, L, M, S, D, Q = 8, 200, 4, 100, 128, 10000
NRG = NPC = NAC = 20
F = 4
NTOT = 300
LC = L // N_CORES              # 25 l-positions per core
R = B * LC                     # 200 rows per core in the l-sharded stages
NS = 13                        # slot capacity per core (cores 4-7 use 12)
OFFS = [0, 13, 26, 39, 52, 64, 76, 88, 100]
BLK = 4                        # slots per PSUM tile
NTILE = 4                      # ceil(13/4)
NCOL = 128                     # columns per PSUM tile
NPAIR = 20                     # (g, a) matmul pairs per path
H_ROWS = 225                   # 25 rgsum + 100 pc + 100 ac
KC0, KC1 = 125, 100            # K-chunks (seg-aligned: 5*25, 4*25)
WSLOT = 384                    # wpre_sh rows per slot: 128 Wp1 + 225 Wp2' + 31 pad
PT_ROWS = N_CORES * 2 * R      # 3200 (both paths)
PT_FULL = 3328                 # 26*128; rows >= PT_ROWS are dustbin
NSCAT = NTILE * NCOL           # 512 staged rows per path
ISQD = float(1.0 / np.sqrt(D))

# pair table: (g, a). a=0: home tile g//4; a=1: home tile g//4-1.
PAIRS = [(g, 0) for g in range(NS)] + \
        [(g, 1) for g in range(NS) if g // BLK >= 1 and g % BLK <= 2]
assert len(PAIRS) == NPAIR
PAIR_TILE = [(g // BLK - a) for (g, a) in PAIRS]
# emission order: all pairs of tile 0, then tile 1, ... (PSUM-group consecutive)
PAIR_ORDER = [i for tt in range(NTILE) for i in range(NPAIR) if PAIR_TILE[i] == tt]

_CACHE = {}


def _wrap_idx(idx, n_pad):
    """int16 index list -> [128, n_pad//16] wrapped/replicated layout."""
    assert len(idx) <= n_pad
    a = np.full(n_pad, -1, np.int16)
    a[:len(idx)] = idx
    a = a.reshape(n_pad // 16, 16).T          # [16, n/16]
    return np.tile(a, (8, 1)).astype(np.int16)  # replicate to 128 partitions


def _host_prep(inp):
    """Build the per-core input maps (pure numpy index/layout prep)."""
    E_skill = np.asarray(inp['E_skill'], np.float32)
    W_cemb = np.asarray(inp['W_cemb'], np.float32)
    W_pre = np.asarray(inp['W_pre'], np.float32).reshape(S, D + NTOT, D)
    E_q = np.ascontiguousarray(np.asarray(inp['E_question'], np.float32))
    E_r = np.asarray(inp['E_response'], np.float32)

    # W'_cemb = [rgsum(25); pc rows; ac rows]  [225, 128]
    wc_rg = W_cemb[0:100].reshape(F, 25, D).sum(0)
    wc_p = np.concatenate([wc_rg, W_cemb[100:200], W_cemb[200:300]], 0)
    wcembT = np.zeros((D, 256), np.float32)
    wcembT[:, :H_ROWS] = wc_p.T

    paths = []
    for pi, (cn, rn, pn, an) in enumerate([
            ('c', 'rgaps', 'pcounts', 'acounts'),
            ('shift_c', 'shft_rgaps', 'shft_pcounts', 'shft_acounts')]):
        paths.append((np.asarray(inp[cn], np.int64),
                      np.asarray(inp[rn], np.int64),
                      np.asarray(inp[pn], np.int64),
                      np.asarray(inp[an], np.int64)))

    in_maps = []
    for k in range(N_CORES):
        s_lo, s_hi = OFFS[k], OFFS[k + 1]
        nsk = s_hi - s_lo
        im = {}
        # --- wpre shard [13, 384, 128] ---
        wsh = np.zeros((NS, WSLOT, D), np.float32)
        for i in range(nsk):
            s = s_lo + i
            blk = W_pre[s]                      # [428, 128]
            wsh[i, 0:128] = blk[0:128]
            wp2 = blk[128:428]                  # [300, 128]
            wsh[i, 128:153] = wp2[0:100].reshape(F, 25, D).sum(0)
            wsh[i, 153:253] = wp2[100:200]
            wsh[i, 253:353] = wp2[200:300]
        im['wpre_sh'] = wsh
        im['wcembT'] = wcembT
        esh = np.zeros((D, NS), np.float32)
        esh[:, :nsk] = E_skill[s_lo:s_hi].T
        im['eskT_sh'] = esh

        # --- forget instance assignment for this core ---
        vals = np.full((9, 2, NPAIR, NCOL), -1.0, np.float32)
        scat = np.full((2, NSCAT), PT_ROWS, np.int64)  # dustbin default
        for pi in range(2):
            c, rg, pc, ac = paths[pi]
            valid = (c >= s_lo) & (c < s_hi)
            bi, li, mi = np.nonzero(valid)
            ss = c[bi, li, mi] - s_lo           # slot in [0, nsk)
            # interval per (b,l)
            key = bi * L + li
            order = np.argsort(key, kind='stable')
            # home tile / coloring
            from collections import defaultdict
            inst_by_bl = defaultdict(list)
            for j in order:
                inst_by_bl[key[j]].append(j)
            # one PSUM column per (b,l) row, allocated per home tile for the
            # whole path (the accumulator is only flushed once per path, so
            # columns are never reused)
            nextcol = [0] * NTILE
            bls = sorted(inst_by_bl.keys(),
                         key=lambda kk: min(ss[j] for j in inst_by_bl[kk]))
            color = {}
            home = {}
            for kk in bls:
                js = inst_by_bl[kk]
                lo = min(ss[j] for j in js)
                hi = max(ss[j] for j in js)
                t = lo // BLK
                assert hi - lo <= 3
                col = nextcol[t]
                nextcol[t] += 1
                assert col < NCOL, f"core {k} path {pi} tile {t}: out of columns"
                color[kk] = col
                home[kk] = t
                # scatter target for this column
                b = kk // L
                l = kk % L
                row = (l // LC) * (2 * R) + pi * R + b * LC + (l % LC)
                scat[pi, t * NCOL + col] = row
            # fill vals per instance
            pair_idx = {p: i for i, p in enumerate(PAIRS)}
            for kk in bls:
                for j in inst_by_bl[kk]:
                    g = int(ss[j])
                    a = 0 if g // BLK == home[kk] else 1
                    pidx = pair_idx[(g, a)]
                    col = color[kk]
                    assert vals[0, pi, pidx, col] < 0, "column collision"
                    vals[0, pi, pidx, col] = rg[bi[j], li[j], mi[j]]
                    vals[1:5, pi, pidx, col] = pc[bi[j], li[j], mi[j]]
                    vals[5:9, pi, pidx, col] = ac[bi[j], li[j], mi[j]]
        import ml_dtypes
        im['valsT'] = vals.reshape(9, 2 * NPAIR * NCOL).astype(ml_dtypes.bfloat16)
        im['scat0'] = _wrap_idx(scat[0].astype(np.int16), NSCAT)
        im['scat1'] = _wrap_idx(scat[1].astype(np.int16), NSCAT)

        # --- l-sharded inputs ---
        lsl = slice(k * LC, (k + 1) * LC)
        q = np.asarray(inp['q'], np.int64)[:, lsl].reshape(-1)        # [200] b-major
        sq = np.asarray(inp['shift_q'], np.int64)[:, lsl].reshape(-1)
        qi = np.concatenate([q, sq]).astype(np.int16)
        im['qidx'] = _wrap_idx(qi, 512)
        r = np.asarray(inp['r'], np.int64)[:, lsl].reshape(-1)
        im['r2'] = np.stack([r.astype(np.float32),
                             np.ones(R, np.float32)])                 # [2, 200]
        cc = np.full((256, 8), -5.0, np.float32)
        for pi in range(2):
            c = paths[pi][0][:, lsl]                                  # [B, LC, M]
            cc[:R, pi * 4:(pi + 1) * 4] = c.reshape(R, M).astype(np.float32)
        im['ccolsT'] = cc

        # patt columns
        p0 = np.zeros((KC0, 1), np.float32)
        p0[:, 0] = np.arange(KC0) % 25
        p1 = np.zeros((KC1, 1), np.float32)
        p1[:, 0] = np.arange(KC1) % 25
        im['patt0'] = p0
        im['patt1'] = p1

        # --- replicated params ---
        im['eq'] = E_q
        im['eresp2'] = np.stack([E_r[1] - E_r[0], E_r[0]])            # [2, 128]
        im['eskT'] = np.ascontiguousarray(E_skill.T)                  # [128, 100]
        im['esk'] = E_skill                                           # [100, 128]
        im['wl2'] = np.asarray(inp['W_l2'], np.float32)               # [384,128]
        im['wl3'] = np.asarray(inp['W_l3'], np.float32)
        im['wfc4'] = np.asarray(inp['W_fc4'], np.float32)
        im['wfc5'] = np.asarray(inp['W_fc5'], np.float32)
        im['wfc3'] = np.asarray(inp['W_fc3'], np.float32)             # [256,1]
        for nm, wn in [('bl2c', 'b_l2'), ('bl3c', 'b_l3'),
                       ('bfc4c', 'b_fc4'), ('bfc5c', 'b_fc5'),
                       ('bprec', 'b_pre')]:
            im[nm] = np.asarray(inp[wn], np.float32).reshape(D, 1)
        im['bfc3'] = np.asarray(inp['b_fc3'], np.float32).reshape(1, 1)
        GP = [0, 1, 3, 2]  # gate order [i, f, o, g] for contiguous sigmoids
        for nm, wn in [('wihT_in', 'wih_in'), ('whhT_in', 'whh_in'),
                       ('wihT_fg', 'wih_fg'), ('whhT_fg', 'whh_fg')]:
            w = np.asarray(inp[wn], np.float32).reshape(4, D, D)[GP]
            im[nm] = np.ascontiguousarray(w.reshape(4 * D, D).T)     # [128, 512]
        im['bg_in'] = np.ascontiguousarray(
            (np.asarray(inp['bih_in'], np.float32)
             + np.asarray(inp['bhh_in'], np.float32)).reshape(4, D)[GP].T)  # [128, 4]
        im['bg_fg'] = np.ascontiguousarray(
            (np.asarray(inp['bih_fg'], np.float32)
             + np.asarray(inp['bhh_fg'], np.float32)).reshape(4, D)[GP].T)
        in_maps.append(im)
    return in_maps


def _declare_inputs(nc):
    t = {}
    def di(name, shape, dtype=F32):
        t[name] = nc.dram_tensor(name, list(shape), dtype, kind="ExternalInput")
    di('wpre_sh', (NS, WSLOT, D)); di('wcembT', (D, 256)); di('eskT_sh', (D, NS))
    di('valsT', (9, 2 * NPAIR * NCOL), BF16)
    di('scat0', (128, NSCAT // 16), I16); di('scat1', (128, NSCAT // 16), I16)
    di('qidx', (128, 512 // 16), I16)
    di('r2', (2, R)); di('ccolsT', (256, 8))
    di('patt0', (KC0, 1)); di('patt1', (KC1, 1))
    di('eq', (Q, D)); di('eresp2', (2, D)); di('eskT', (D, S)); di('esk', (S, D))
    di('wl2', (3 * D, D)); di('wl3', (2 * D, D))
    di('wfc4', (2 * D, D)); di('wfc5', (2 * D, D)); di('wfc3', (2 * D, 1))
    for nm in ['bl2c', 'bl3c', 'bfc4c', 'bfc5c', 'bprec']:
        di(nm, (D, 1))
    di('bfc3', (1, 1))
    for nm in ['wihT_in', 'whhT_in', 'wihT_fg', 'whhT_fg']:
        di(nm, (D, 4 * D))
    di('bg_in', (D, 4)); di('bg_fg', (D, 4))
    return t


def _build_module(debug=False):
    nc = bacc.Bacc("TRN2", target_bir_lowering=False, debug=False,
                   num_devices=N_CORES)
    t = _declare_inputs(nc)
    y_out = nc.dram_tensor("y", [1, R], F32, kind="ExternalOutput")
    pt = nc.dram_tensor("pt", [PT_FULL, D], F32)
    ptr = nc.dram_tensor("ptr", [2 * R, D], F32)
    dbg = {}
    if debug:
        for nm, w in [('d_fh', 2 * R), ('d_xemb', R), ('d_att', 2 * R),
                      ('d_ih', R), ('d_fgh', R), ('d_x1', R), ('d_x2', R)]:
            dbg[nm] = nc.dram_tensor(nm, [D, w], F32, kind="ExternalOutput")

    with tile.TileContext(nc) as tc:
        _emit(nc, tc, t, y_out, pt, ptr, dbg)
    nc.compile()
    return nc


def _emit(nc, tc, t, y_out, pt, ptr, dbg=None):
    import contextlib
    ctx = contextlib.ExitStack()
    cp = ctx.enter_context(tc.tile_pool(name="const", bufs=1))
    wp = ctx.enter_context(tc.tile_pool(name="work", bufs=2))
    pp = ctx.enter_context(tc.tile_pool(name="psum", bufs=4, space="PSUM"))
    ppU = ctx.enter_context(tc.tile_pool(name="psumU", bufs=1, space="PSUM"))

    ident = cp.tile([128, 128], F32, tag="ident")
    make_identity(nc, ident[:])
    iotai = cp.tile([128, S], I32, tag="iotai")
    nc.gpsimd.iota(iotai[:], pattern=[[1, S]], base=0, channel_multiplier=0)
    iotf = cp.tile([128, S], F32, tag="iotf")
    nc.vector.tensor_copy(out=iotf[:], in_=iotai[:])

    def load(name, shape=None, dtype=F32, tag=None, eng=None):
        tl = cp.tile(shape or list(t[name].shape), dtype, tag=tag or name)
        (eng or nc.sync).dma_start(out=tl[:], in_=t[name][:])
        return tl

    # --- early: question-embedding gather (SWDGE, independent of SP queue) ---
    qidx = load('qidx', dtype=I16, eng=nc.gpsimd)
    embg = cp.tile([128, 4, D], F32, tag="embg")
    nc.gpsimd.dma_gather(out_ap=embg[:], in_ap=t['eq'][:], idxs_ap=qidx[:],
                         num_idxs=512, num_idxs_reg=2 * R, elem_size=D)

    # --- zero Pt (ACT ring) ---
    zt = cp.tile([128, D], F32, tag="zt")
    nc.vector.memset(zt[:], 0.0)
    ztb = zt[:].rearrange("p (a d) -> p a d", a=1).to_broadcast(
        [128, PT_FULL // 128, D])
    nc.scalar.dma_start(out=pt[:].rearrange("(a p) d -> p a d", p=128), in_=ztb)

    # --- attention/LSTM-path constants (SP ring, first in FIFO) ---
    eskT = load('eskT')
    esk = load('esk')
    eresp2 = load('eresp2')
    r2 = load('r2')
    ccolsT = cp.tile([128, 2, 8], F32, tag="ccolsT")
    nc.sync.dma_start(out=ccolsT[:],
                      in_=t['ccolsT'][:].rearrange("(a p) c -> p a c", p=128))

    def load_kchunks(name, nchunk, ncol=D, eng=None):
        tl = cp.tile([128, nchunk, ncol], F32, tag=name)
        (eng or nc.sync).dma_start(
            out=tl[:], in_=t[name][:].rearrange("(a p) d -> p a d", p=128))
        return tl
    wl2 = load_kchunks('wl2', 3)
    wl3 = load_kchunks('wl3', 2)
    wfc4 = load_kchunks('wfc4', 2)
    wfc5 = load_kchunks('wfc5', 2)
    wfc3 = load_kchunks('wfc3', 2, ncol=1)
    bl2c = load('bl2c'); bl3c = load('bl3c')
    bfc4c = load('bfc4c'); bfc5c = load('bfc5c'); bprec = load('bprec')
    bfc3 = load('bfc3')
    wihT_in = load('wihT_in'); whhT_in = load('whhT_in')
    wihT_fg = load('wihT_fg'); whhT_fg = load('whhT_fg')
    bg_in = load('bg_in'); bg_fg = load('bg_fg')

    # --- forget-path loads: spread across ACT HWDGE + SWDGE rings ---
    wcembT = load('wcembT', eng=nc.scalar)
    eskT_sh = load('eskT_sh', eng=nc.scalar)
    scat_idx = [load('scat0', dtype=I16, eng=nc.gpsimd),
                load('scat1', dtype=I16, eng=nc.gpsimd)]
    wsh = t['wpre_sh']
    wp1 = cp.tile([128, NS, 128], F32, tag="wp1")
    nc.gpsimd.dma_start(out=wp1[:], in_=wsh[:].rearrange("s p d -> p s d")[0:128])
    wp2a = cp.tile([KC0, NS, 128], F32, tag="wp2a")
    nc.gpsimd.dma_start(out=wp2a[:],
                        in_=wsh[:].rearrange("s p d -> p s d")[128:128 + KC0])
    wp2b = cp.tile([KC1, NS, 128], F32, tag="wp2b")
    nc.gpsimd.dma_start(
        out=wp2b[:],
        in_=wsh[:].rearrange("s p d -> p s d")[128 + KC0:128 + H_ROWS])
    valt0 = cp.tile([KC0, 2 * NPAIR * NCOL], BF16, tag="valt0")
    valt1 = cp.tile([KC1, 2 * NPAIR * NCOL], BF16, tag="valt1")
    for j in range(5):
        eng = nc.sync if j % 2 == 0 else nc.scalar
        eng.dma_start(out=valt0[25 * j:25 * (j + 1), :],
                      in_=t['valsT'][j:j + 1, :].to_broadcast(
                          [25, 2 * NPAIR * NCOL]))
    for j in range(4):
        eng = nc.scalar if j % 2 == 0 else nc.sync
        eng.dma_start(out=valt1[25 * j:25 * (j + 1), :],
                      in_=t['valsT'][5 + j:6 + j, :].to_broadcast(
                          [25, 2 * NPAIR * NCOL]))

    # ================ l-sharded path (emitted early for overlap) ===========
    embT = cp.tile([128, 512], F32, tag="embT")
    for jj in range(4):
        tps = pp.tile([128, 128], F32, tag="ps")
        nc.tensor.transpose(out=tps[:], in_=embg[:, jj, :], identity=ident[:])
        nc.vector.tensor_copy(out=embT[:, 128 * jj:128 * (jj + 1)], in_=tps[:])

    embrT = cp.tile([128, R], F32, tag="embrT")
    psr = pp.tile([128, R], F32, tag="ps")
    nc.tensor.matmul(out=psr[:], lhsT=eresp2[:], rhs=r2[:], start=True, stop=True)
    nc.vector.tensor_copy(out=embrT[:], in_=psr[:])

    attT = {}
    for pi in range(2):
        qT = embT[:, pi * R:(pi + 1) * R]
        wexpT = wp.tile([S, R], F32, tag=f"wexpT{pi}", name=f"wexpT{pi}")
        for ch, (r0, nr) in enumerate([(0, 128), (128, 72)]):
            scps = pp.tile([128, S], F32, tag="ps")
            nc.tensor.matmul(out=scps[:nr], lhsT=qT[:, r0:r0 + nr],
                             rhs=eskT[:], start=True, stop=True)
            cv1 = wp.tile([128, S], F32, tag="cv1")
            for mm in range(M):
                cm = ccolsT[:nr, ch, 4 * pi + mm:4 * pi + mm + 1]
                if mm == 0:
                    nc.vector.tensor_scalar(out=cv1[:nr], in0=iotf[:nr],
                                            scalar1=cm, scalar2=None,
                                            op0=ALU.is_equal)
                else:
                    tmpm = wp.tile([128, S], F32, tag="tmpm")
                    nc.vector.tensor_scalar(out=tmpm[:nr], in0=iotf[:nr],
                                            scalar1=cm, scalar2=None,
                                            op0=ALU.is_equal)
                    nc.vector.tensor_tensor(out=cv1[:nr], in0=cv1[:nr],
                                            in1=tmpm[:nr], op=ALU.add)
            sc = wp.tile([128, S], F32, tag="sc")
            nc.vector.tensor_tensor(out=sc[:nr], in0=scps[:nr], in1=cv1[:nr],
                                    op=ALU.mult)
            ex = wp.tile([128, S], F32, tag="ex")
            nc.scalar.activation(out=ex[:nr], in_=sc[:nr], func=AF.Exp,
                                 scale=ISQD)
            den = wp.tile([128, 1], F32, tag="den")
            nc.vector.tensor_reduce(out=den[:nr], in_=ex[:nr],
                                    axis=mybir.AxisListType.X, op=ALU.add)
            rec = wp.tile([128, 1], F32, tag="rec")
            nc.vector.reciprocal(out=rec[:nr], in_=den[:nr])
            wexp = wp.tile([128, S], F32, tag="wexp")
            nc.vector.tensor_tensor(out=wexp[:nr], in0=ex[:nr], in1=cv1[:nr],
                                    op=ALU.mult)
            nc.vector.tensor_scalar(out=wexp[:nr], in0=wexp[:nr],
                                    scalar1=rec[:nr, 0:1], scalar2=None,
                                    op0=ALU.mult)
            wps = pp.tile([S, 128], F32, tag="ps")
            nc.tensor.transpose(out=wps[:, :nr], in_=wexp[:nr],
                                identity=ident[:nr, :nr])
            nc.vector.tensor_copy(out=wexpT[:, r0:r0 + nr], in_=wps[:, :nr])
        aT = cp.tile([128, R], F32, tag=f"attT{pi}", name=f"attT{pi}")
        aps = pp.tile([128, R], F32, tag="ps")
        nc.tensor.matmul(out=aps[:], lhsT=esk[:], rhs=wexpT[:], start=True,
                         stop=True)
        nc.vector.tensor_copy(out=aT[:], in_=aps[:])
        attT[pi] = aT

    xembT = cp.tile([128, R], F32, tag="xembT")
    xps = pp.tile([128, R], F32, tag="ps")
    nc.tensor.matmul(out=xps[:], lhsT=wl2[:, 0, :], rhs=embT[:, 0:R],
                     start=True, stop=False)
    nc.tensor.matmul(out=xps[:], lhsT=wl2[:, 1, :], rhs=embrT[:],
                     start=False, stop=False)
    nc.tensor.matmul(out=xps[:], lhsT=wl2[:, 2, :], rhs=attT[0][:],
                     start=False, stop=True)
    nc.vector.tensor_scalar(out=xembT[:], in0=xps[:], scalar1=bl2c[:, 0:1],
                            scalar2=None, op0=ALU.add)

    shiftT = cp.tile([128, R], F32, tag="shiftT")
    sps = pp.tile([128, R], F32, tag="ps")
    nc.tensor.matmul(out=sps[:], lhsT=wl3[:, 0, :], rhs=embT[:, R:2 * R],
                     start=True, stop=False)
    nc.tensor.matmul(out=sps[:], lhsT=wl3[:, 1, :], rhs=attT[1][:],
                     start=False, stop=True)
    nc.vector.tensor_scalar(out=shiftT[:], in0=sps[:], scalar1=bl3c[:, 0:1],
                            scalar2=None, op0=ALU.add)

    if dbg:
        nc.sync.dma_start(out=dbg['d_xemb'][:], in_=xembT[:])
        nc.sync.dma_start(out=dbg['d_att'][:, 0:R], in_=attT[0][:])
        nc.sync.dma_start(out=dbg['d_att'][:, R:2 * R], in_=attT[1][:])

    # ---------------- LSTM (scan over B, batch = LC) ----------------
    hzero = cp.tile([128, LC], F32, tag="hzero")
    nc.vector.memset(hzero[:], 0.0)

    def lstm(xT, wihT, whhT, bg, name):
        """xT [128, 200] (cols b-major). Gates host-ordered [i, f, o, g]."""
        gih = cp.tile([128, 4, R], F32, tag=f"gih_{name}", name=f"gih_{name}")
        for gi in range(4):
            gps = pp.tile([128, R], F32, tag="ps")
            nc.tensor.matmul(out=gps[:], lhsT=wihT[:, gi * 128:(gi + 1) * 128],
                             rhs=xT[:], start=True, stop=True)
            nc.vector.tensor_scalar(out=gih[:, gi, :], in0=gps[:],
                                    scalar1=bg[:, gi:gi + 1], scalar2=None,
                                    op0=ALU.add)
        hT = cp.tile([128, R], F32, tag=f"hT_{name}", name=f"hT_{name}")
        cs = cp.tile([128, LC], F32, tag=f"cs_{name}", name=f"cs_{name}")
        for b in range(B):
            sl = slice(b * LC, (b + 1) * LC)
            hprev = hzero[:] if b == 0 else hT[:, (b - 1) * LC:b * LC]
            gps = pp.tile([128, 4, LC], F32, tag="ps")
            for gi in range(4):
                nc.tensor.matmul(out=gps[:, gi, :],
                                 lhsT=whhT[:, gi * 128:(gi + 1) * 128],
                                 rhs=hprev, start=True, stop=True)
            gt = wp.tile([128, 4, LC], F32, tag=f"gt_{name}")
            nc.vector.tensor_tensor(out=gt[:], in0=gps[:],
                                    in1=gih[:, :, sl], op=ALU.add)
            sig = wp.tile([128, 4, LC], F32, tag=f"sig_{name}")
            nc.scalar.activation(out=sig[:, 0:3, :], in_=gt[:, 0:3, :],
                                 func=AF.Sigmoid)
            nc.scalar.activation(out=sig[:, 3, :], in_=gt[:, 3, :], func=AF.Tanh)
            if b > 0:
                cs2 = wp.tile([128, LC], F32, tag=f"cs2_{name}")
                nc.vector.tensor_tensor(out=cs2[:], in0=sig[:, 1, :], in1=cs[:],
                                        op=ALU.mult)
            it = wp.tile([128, LC], F32, tag=f"it_{name}")
            nc.vector.tensor_tensor(out=it[:], in0=sig[:, 0, :],
                                    in1=sig[:, 3, :], op=ALU.mult)
            if b > 0:
                nc.vector.tensor_tensor(out=cs[:], in0=cs2[:], in1=it[:],
                                        op=ALU.add)
            else:
                nc.vector.tensor_copy(out=cs[:], in_=it[:])
            tc_t = wp.tile([128, LC], F32, tag=f"tc_{name}")
            nc.scalar.activation(out=tc_t[:], in_=cs[:], func=AF.Tanh)
            nc.vector.tensor_tensor(out=hT[:, sl], in0=sig[:, 2, :],
                                    in1=tc_t[:], op=ALU.mult)
        return hT

    # ================ forget pipeline ================
    a_sb = cp.tile([128, NS, 128], F32, tag="a_sb")
    for i in range(NS):
        nc.vector.tensor_scalar(
            out=a_sb[:, i, :], in0=wp1[:, i, :],
            scalar1=eskT_sh[:, i:i + 1], scalar2=None, op0=ALU.mult)
    h0 = cp.tile([KC0, NS, 128], F32, tag="h0")
    h1 = cp.tile([KC1, NS, 128], F32, tag="h1")
    for i in range(NS):
        ps0 = pp.tile([KC0, 128], F32, tag="ps")
        nc.tensor.matmul(out=ps0[:], lhsT=wcembT[:, 0:KC0], rhs=a_sb[:, i, :],
                         start=True, stop=True)
        nc.vector.tensor_tensor(out=h0[:, i, :], in0=ps0[:], in1=wp2a[:, i, :],
                                op=ALU.add)
        ps1 = pp.tile([KC1, 128], F32, tag="ps")
        nc.tensor.matmul(out=ps1[:], lhsT=wcembT[:, KC0:H_ROWS],
                         rhs=a_sb[:, i, :], start=True, stop=True)
        nc.vector.tensor_tensor(out=h1[:, i, :], in0=ps1[:], in1=wp2b[:, i, :],
                                op=ALU.add)

    oht0 = cp.tile([KC0, 2 * NPAIR * NCOL], F32, tag="oht0")
    oht1 = cp.tile([KC1, 2 * NPAIR * NCOL], F32, tag="oht1")
    patt0 = load('patt0', eng=nc.scalar)
    patt1 = load('patt1', eng=nc.scalar)
    nc.vector.tensor_scalar(out=oht0[:], in0=valt0[:], scalar1=patt0[:, 0:1],
                            scalar2=None, op0=ALU.is_equal)
    nc.gpsimd.tensor_scalar(out=oht1[:], in0=valt1[:], scalar1=patt1[:, 0:1],
                            scalar2=None, op0=ALU.is_equal)

    u_ps = [ppU.tile([NCOL, NTILE, 128], F32, tag=f"u{pi}", name=f"u{pi}")
            for pi in range(2)]
    stag = [wp.tile([128, NTILE, 128], F32, tag=f"stag{pi}", name=f"stag{pi}")
            for pi in range(2)]
    for pi in range(2):
        for j, i in enumerate(PAIR_ORDER):
            g, a = PAIRS[i]
            tt = PAIR_TILE[i]
            col0 = (pi * NPAIR + i) * NCOL
            first = (j == 0 or PAIR_TILE[PAIR_ORDER[j - 1]] != tt)
            last = (j == NPAIR - 1 or PAIR_TILE[PAIR_ORDER[j + 1]] != tt)
            nc.tensor.matmul(out=u_ps[pi][:, tt, :],
                             lhsT=oht0[:, col0:col0 + NCOL],
                             rhs=h0[:, g, :], start=first, stop=False,
                             skip_group_check=True)
            nc.tensor.matmul(out=u_ps[pi][:, tt, :],
                             lhsT=oht1[:, col0:col0 + NCOL],
                             rhs=h1[:, g, :], start=False, stop=last,
                             skip_group_check=True)
        nc.vector.tensor_copy(out=stag[pi][:], in_=u_ps[pi][:])
        nc.gpsimd.dma_scatter_add(
            out_ap=pt[:], in_ap=stag[pi][:], idxs_ap=scat_idx[pi][:],
            num_idxs=NSCAT, num_idxs_reg=NSCAT, elem_size=D)
    nc.gpsimd.collective_compute(
        "ReduceScatter", ALU.add,
        replica_groups=[list(range(N_CORES))],
        ins=[pt[0:PT_ROWS, :]], outs=[ptr[:]])

    inputhT = lstm(xembT, wihT_in, whhT_in, bg_in, "in")

    # fhT [128, 400]: cols 0:200 = forget path, 200:400 = shift path (b-major)
    fhT = cp.tile([128, 2 * R], F32, tag="fhT")
    for jj in range(4):
        pc_t = wp.tile([100, D], F32, tag="ptr_l")
        nc.sync.dma_start(out=pc_t[:], in_=ptr[100 * jj:100 * (jj + 1), :])
        tps = pp.tile([128, 100], F32, tag="ps")
        nc.tensor.transpose(out=tps[:], in_=pc_t[:], identity=ident[:100, :100])
        nc.vector.tensor_scalar(out=fhT[:, 100 * jj:100 * (jj + 1)], in0=tps[:],
                                scalar1=bprec[:, 0:1], scalar2=None, op0=ALU.add)
    if dbg:
        nc.sync.dma_start(out=dbg['d_fh'][:], in_=fhT[:])

    forgethT = lstm(fhT[:, 0:R], wihT_fg, whhT_fg, bg_fg, "fg")
    if dbg:
        nc.sync.dma_start(out=dbg['d_ih'][:], in_=inputhT[:])
        nc.sync.dma_start(out=dbg['d_fgh'][:], in_=forgethT[:])

    # ---------------- heads ----------------
    x1T = cp.tile([128, R], F32, tag="x1T")
    h1ps = pp.tile([128, R], F32, tag="ps")
    nc.tensor.matmul(out=h1ps[:], lhsT=wfc4[:, 0, :], rhs=shiftT[:],
                     start=True, stop=False)
    nc.tensor.matmul(out=h1ps[:], lhsT=wfc4[:, 1, :], rhs=inputhT[:],
                     start=False, stop=True)
    nc.scalar.activation(out=x1T[:], in_=h1ps[:], func=AF.Relu,
                         bias=bfc4c[:, 0:1])
    x2T = cp.tile([128, R], F32, tag="x2T")
    h2ps = pp.tile([128, R], F32, tag="ps")
    nc.tensor.matmul(out=h2ps[:], lhsT=wfc5[:, 0, :], rhs=fhT[:, R:2 * R],
                     start=True, stop=False)
    nc.tensor.matmul(out=h2ps[:], lhsT=wfc5[:, 1, :], rhs=forgethT[:],
                     start=False, stop=True)
    nc.scalar.activation(out=x2T[:], in_=h2ps[:], func=AF.Relu,
                         bias=bfc5c[:, 0:1])
    if dbg:
        nc.sync.dma_start(out=dbg['d_x1'][:], in_=x1T[:])
        nc.sync.dma_start(out=dbg['d_x2'][:], in_=x2T[:])

    yps = pp.tile([1, R], F32, tag="ps")
    nc.tensor.matmul(out=yps[:], lhsT=wfc3[:, 0, :], rhs=x1T[:],
                     start=True, stop=False)
    nc.tensor.matmul(out=yps[:], lhsT=wfc3[:, 1, :], rhs=x2T[:],
                     start=False, stop=True)
    ysb = wp.tile([1, R], F32, tag="ysb")
    nc.scalar.activation(out=ysb[:], in_=yps[:], func=AF.Sigmoid,
                         bias=bfc3[0:1, 0:1])
    nc.sync.dma_start(out=y_out[:], in_=ysb[:])
    ctx.close()


def get_module(debug=False):
    key = ('ncd' if debug else 'nc')
    if key not in _CACHE:
        _CACHE[key] = _build_module(debug)
    return _CACHE[key]


def kernel_debug(**inputs):
    nc = get_module(debug=True)
    in_maps = _host_prep(inputs)
    res = run_bass_kernel_spmd(nc, in_maps, list(range(N_CORES)), trace=False)
    return res.results


def kernel(**inputs):
    nc = get_module()
    in_maps = _host_prep(inputs)
    res = run_bass_kernel_spmd(nc, in_maps, list(range(N_CORES)), trace=False)
    y = np.zeros((B, L), np.float32)
    for k in range(N_CORES):
        y[:, k * LC:(k + 1) * LC] = res.results[k]['y'].reshape(B, LC)
    return y
